# revision 35
# baseline (speedup 1.0000x reference)
"""GQA attention (RoPE, full softmax) on 8 TRN2 NeuronCores — v4.

Tensor-parallel over heads: core g owns KV head g and Q heads 4g..4g+3.
Each core computes y_g = concat_h(softmax(Q_h K^T) V_h) @ wo_h^T; the host
sums the 8 bf16 partials in fp32.

v4 changes vs v3 (all driven by TimelineSim engine-occupancy analysis):
  - softmax row sums moved OFF the PE: P tiles are accumulated with
    tensor_add alternating between DVE (even kt) and GpSimd/Pool (odd kt),
    then one Pool partition_all_reduce + DVE reciprocal.  This deletes the
    256 ones-matmuls (~57us of PE busy) and their PSUM banks.
  - PSUM: S double-buffered (2x2 banks) + PV accumulator double-buffered
    (2x2 banks) so consecutive (chunk-pair, head) blocks overlap the
    softmax-normalization tail.
  - phase A post-pass: all PSUM->SBUF drains are emitted FIRST (split
    ACT/DVE), then the halves-swap DMAs, then the rope mul chains, so the
    next pass's (and phase B's) PSUM allocation isn't serialized behind
    the whole DVE rope chain.
  - startup: x/weight DMAs interleaved in small chunks so the first
    matmul starts ~1.5us in instead of ~8us.
  - last output block stores y in two half-DMAs to shorten the end drain.

fp32 PSUM accumulation everywhere; bf16 inputs/activations keep rel err
well under the 2e-2 gate.  exp() without max-subtraction is safe:
scores ~ N(0,1), |s|max ~ 8.
"""

import numpy as np

import concourse.bass as bass
import concourse.bass_isa as bass_isa
import concourse.mybir as mybir
import concourse.tile as tile
from concourse import bacc
from concourse.bass_utils import run_bass_kernel_spmd
from concourse.masks import make_identity

F32 = mybir.dt.float32
BF16 = mybir.dt.bfloat16
EXP = mybir.ActivationFunctionType.Exp
COPY = mybir.ActivationFunctionType.Copy

DIM, N_HEADS, N_KV_HEADS, HEAD_DIM, SEQ = 4096, 32, 8, 128, 2048
CORES = 8
QH = N_HEADS // CORES  # q heads per core
CHS = 512              # q-chunk size (1 PSUM bank of fp32)
RES = 17               # x c-tiles kept SBUF-resident across phase-A passes


def _body(tc, xT, wqkv, woT, csn, y, dim, seq, qh, phases=3):
    nc = tc.nc
    CT = dim // 128   # contraction tiles over model dim
    KT = seq // 128   # key tiles
    CH = seq // CHS   # q chunks
    HD = HEAD_DIM
    scale = HD ** -0.5

    with tc.tile_pool(name="persist", bufs=1) as persist:
        QT = [persist.tile([128, seq], BF16, name=f"qt{h}", tag=f"qt{h}") for h in range(qh)]
        KTs = persist.tile([128, seq], BF16, tag="kts")
        Vs = persist.tile([128, KT, HD], BF16, tag="vs")
        ident = persist.tile([128, 128], F32, tag="ident")
        make_identity(nc, ident)

        # ------------- Phase A: QKV projections + RoPE + V transpose -------------
        # passes: 0 -> {k,v}, 1 -> {q0,q1}, 2 -> {q2,q3}
        with (
            tc.tile_pool(name="xres", bufs=1) as xres,
            tc.tile_pool(name="wqa", bufs=1) as wqa,
            tc.tile_pool(name="csnp", bufs=1) as csnp,
            tc.tile_pool(name="xs", bufs=4) as xs,
            tc.tile_pool(name="stage", bufs=1) as stg,
            tc.tile_pool(name="rope", bufs=2) as rp,
            tc.tile_pool(name="pps", bufs=8, space="PSUM") as pps,
        ):
            cs_t = csnp.tile([128, seq], BF16, tag="cs")
            sn_t = csnp.tile([128, seq], BF16, tag="sn")
            xresid = xres.tile([128, RES, seq], BF16, tag="xres")
            wqall = wqa.tile([128, CT, (qh + 2) * HD], BF16, tag="wqall")
            KVC = slice(qh * HD, (qh + 2) * HD)   # K,V weight columns

            def w_dma(csl, nsl):
                nc.sync.dma_start(
                    out=wqall[:, csl, nsl],
                    in_=wqkv[csl, :, nsl].rearrange("c p n -> p c n"),
                )

            # startup ramp: pass 0 only needs the K/V weight columns (2.1MB
            # instead of 6.3MB) — pass 0 is DMA-bandwidth-bound (16MB of x),
            # so Q weights + cos/sin stream during pass 1 instead.
            # first x tile arrives in j-sized chunks: matmul (c=0, j=0) only
            # needs columns 0:512, so it starts ~2us earlier
            nc.sync.dma_start(out=xresid[:, 0, 0:CHS], in_=xT[0:128, 0:CHS])
            w_dma(slice(0, 1), KVC)
            nc.sync.dma_start(out=xresid[:, 0, CHS:], in_=xT[0:128, CHS:])
            nc.sync.dma_start(out=xresid[:, 1, :], in_=xT[128:256, :])
            w_dma(slice(1, 4), KVC)
            nc.sync.dma_start(out=xresid[:, 2, :], in_=xT[256:384, :])
            w_dma(slice(4, 8), KVC)

            def drain(p, t, j, ps, tsbs, vts):
                """PSUM->SBUF drain for out-tile (t, j), emitted inline right
                after its final (stop=True) matmul so banks free during the
                last c-iteration instead of after the pass."""
                if p == 0 and t == 1:
                    nc.scalar.activation(out=vts[j], in_=ps[1][j], func=COPY)
                elif p == 1 and t == 1:
                    nc.scalar.activation(out=tsbs[j][:, 1, :], in_=ps[1][j], func=COPY)
                else:
                    # pass 2 drains all on DVE so phase B's first exp isn't
                    # queued behind ACT drains
                    nc.vector.tensor_copy(out=tsbs[j][:, t, :], in_=ps[t][j])

            def post_pass(p, ps, tsbs, vts):
                """Swaps, V transposes, then the rope mul chains (the PSUM
                drains already ran inline in the last c-iteration)."""
                sws = []
                m = 1 if p == 0 else 2
                for j in range(CH):
                    sw = stg.tile([128, 2, CHS], BF16, name=f"sw{j}", tag=f"sw{j}")
                    sws.append(sw)
                    nc.sync.dma_start(out=sw[0:64, 0:m], in_=tsbs[j][64:128, 0:m])
                    nc.sync.dma_start(out=sw[64:128, 0:m], in_=tsbs[j][0:64, 0:m])
                if p == 0:
                    # V transpose: [d, keys] -> [keys, d] per 128-key block
                    for j in range(CH):
                        for b in range(CHS // HD):
                            trp = pps.tile([128, HD], F32, name="trp", tag="mm")
                            nc.tensor.transpose(
                                trp, vts[j][:, b * HD:(b + 1) * HD], ident
                            )
                            nc.vector.tensor_copy(
                                out=Vs[:, j * (CHS // HD) + b, :], in_=trp
                            )
                for j in range(CH):
                    jsl = slice(j * CHS, (j + 1) * CHS)
                    outs = (
                        [KTs[:, jsl]] if p == 0
                        else [QT[2 * (p - 1)][:, jsl], QT[2 * p - 1][:, jsl]]
                    )
                    for i, out in enumerate(outs):
                        # all-bf16 operands keep these on the DVE 2x/4x path
                        t1 = rp.tile([128, CHS], BF16, tag="t1")
                        t2 = rp.tile([128, CHS], BF16, tag="t2")
                        nc.vector.tensor_mul(t1, tsbs[j][:, i, :], cs_t[:, jsl])
                        nc.vector.tensor_mul(t2, sws[j][:, i, :], sn_t[:, jsl])
                        nc.vector.tensor_add(out, t1, t2)

            for p in range(3):
                ps = [
                    [
                        pps.tile([128, CHS], F32, name=f"mm{t}{j}", tag="mm")
                        for j in range(CH)
                    ]
                    for t in range(2)
                ]
                for c in range(CT):
                    if p == 0:
                        # remaining K/V weight c-tiles mid-pass; the first Q
                        # c-tiles late in the pass (pass 1 needs them at its
                        # c=0); everything else during pass 1, whose DMA
                        # queue is nearly idle until c reaches RES
                        if c == 6:
                            w_dma(slice(8, 20), KVC)
                        elif c == 14:
                            w_dma(slice(20, CT), KVC)
                        elif c >= 28:
                            w_dma(slice(c - 28, c - 27), slice(0, 2 * HD))
                    if p == 1:
                        if c == 0:
                            w_dma(slice(4, 8), slice(0, 2 * HD))
                        elif c == 2:
                            w_dma(slice(8, 16), slice(0, 2 * HD))
                        elif c == 4:
                            w_dma(slice(16, 24), slice(0, 2 * HD))
                        elif c == 6:
                            w_dma(slice(24, CT), slice(0, 2 * HD))
                        elif c == 8:
                            w_dma(slice(0, 16), slice(2 * HD, 4 * HD))
                        elif c == 16:
                            w_dma(slice(16, CT), slice(2 * HD, 4 * HD))
                    if c < RES:
                        xt = xresid[:, c, :]
                        if p == 0 and c >= 3:
                            nc.sync.dma_start(out=xt, in_=xT[c * 128:(c + 1) * 128, :])
                    else:
                        xt = xs.tile([128, seq], BF16, name="xt", tag="xs")
                        nc.sync.dma_start(out=xt, in_=xT[c * 128:(c + 1) * 128, :])
                    if c == CT - 1:
                        tsbs = [
                            stg.tile([128, 2, CHS], BF16, name=f"tsb{j}", tag=f"tsb{j}")
                            for j in range(CH)
                        ]
                        vts = (
                            [
                                stg.tile([128, CHS], F32, name=f"vt{j}", tag=f"vt{j}")
                                for j in range(CH)
                            ]
                            if p == 0
                            else None
                        )
                    for t in range(2):
                        tile_idx = (4 + t) if p == 0 else (2 * (p - 1) + t)
                        w_sl = slice(tile_idx * HD, (tile_idx + 1) * HD)
                        for j in range(CH):
                            nc.tensor.matmul(
                                ps[t][j],
                                lhsT=wqall[:, c, w_sl],
                                rhs=xt[:, j * CHS:(j + 1) * CHS],
                                start=(c == 0),
                                stop=(c == CT - 1),
                            )
                            if c == CT - 1:
                                drain(p, t, j, ps, tsbs, vts)
                if p == 0:
                    # MUST be emitted before post_pass(0)'s rope muls read
                    # them (consumers emitted before producers get no
                    # dependency); queued after pass 0's x streams so the
                    # pass-0 DMA budget is unaffected
                    nc.sync.dma_start(out=cs_t, in_=csn[0])
                    nc.sync.dma_start(out=sn_t, in_=csn[1])
                post_pass(p, ps, tsbs, vts)

        if phases == 1:
            nc.sync.dma_start(out=y[0:128, 0:seq], in_=KTs)
            return

        # ---------------- Phase B: attention per (chunk-pair, head) ----------------
        # NOTE: the B pools stay open through phase C — a pool release
        # boundary depends on EVERY user of the pool, so closing them would
        # serialize C's PSUM allocation behind B's whole softmax tail.
        # Phase C instead allocates its PSUM banks from the same st/opst
        # tags, whose WAR dependencies resolve tile-by-tile.
        # pool order matters: pp/sm open FIRST so they land on the SBUF
        # regions freed by xres/wqa (whose last users are plain matmuls,
        # done at phase-A end) — NOT on the stage/rope regions whose
        # release waits for the whole pass-2 rope chain on DVE.  wo/OT/ysb
        # land on the rope-dependent regions; their first uses are late.
        with (
            tc.tile_pool(name="pp", bufs=2) as pp,
            tc.tile_pool(name="sm", bufs=2) as sm,
            tc.tile_pool(name="sps", bufs=2, space="PSUM") as sps,
            tc.tile_pool(name="aps", bufs=2, space="PSUM") as aps,
            tc.tile_pool(name="wo", bufs=1) as wop,
            tc.tile_pool(name="otp", bufs=1) as otp,
            tc.tile_pool(name="ysb", bufs=3) as ysb,
        ):
            # preload wo during phase B (used in phase C)
            wo_r = wop.tile([128, qh, dim], BF16)
            for h in range(qh):
                nc.sync.dma_start(out=wo_r[:, h, :], in_=woT[h])
            OT = [otp.tile([128, seq], BF16, name=f"ot{h}", tag=f"ot{h}") for h in range(qh)]
            if True:
                pending_tail = None
                for jp in range(CH // 2):
                    for h in range(qh):
                        pt = pp.tile([128, KT, 2, CHS], BF16, name="pt", tag="pt")
                        opst = aps.tile([128, 2, CHS], F32, name="opst", tag="opst")
                        accd = sm.tile([128, 2, CHS], F32, tag="accd")
                        accp = sm.tile([128, 2, CHS], F32, tag="accp")

                        def acc_p(k, pt=pt, accd=accd, accp=accp):
                            # row-sum accumulation off the PE.  Pool adds are
                            # ~2.1us/[128,1024] (software engine) vs DVE's
                            # ~0.85us, so Pool only takes kt 3 and 9 — enough
                            # to leave Pool idle at block end (the all_reduce
                            # starts immediately) with 2x margin against
                            # cost-model error on the software engine.
                            pool = k in (3, 6, 9, 12)
                            eng = nc.gpsimd if pool else nc.vector
                            acc = accp if pool else accd
                            first = k == 3 if pool else k == 0
                            if first:
                                eng.tensor_copy(out=acc, in_=pt[:, k])
                            else:
                                eng.tensor_add(acc, acc, pt[:, k])

                        for kt in range(KT):
                            st = sps.tile([128, 2, CHS], F32, name="st", tag="st")
                            for i in range(2):
                                j = 2 * jp + i
                                nc.tensor.matmul(
                                    st[:, i, :],
                                    lhsT=KTs[:, kt * 128:(kt + 1) * 128],
                                    rhs=QT[h][:, j * CHS:(j + 1) * CHS],
                                    start=True,
                                    stop=True,
                                )
                            nc.scalar.activation(
                                out=pt[:, kt], in_=st, func=EXP, scale=scale
                            )
                            # PV lags TWO kt so the in-order PE queue never
                            # sees even the semaphore-settle latency of
                            # exp(kt) (~100-200ns per kt otherwise)
                            if kt > 1:
                                for i in range(2):
                                    nc.tensor.matmul(
                                        opst[:, i, :],
                                        lhsT=Vs[:, kt - 2, :],
                                        rhs=pt[:, kt - 2, i, :],
                                        start=(kt - 2 == 0),
                                        stop=False,
                                    )
                            if kt > 0:
                                acc_p(kt - 1)
                            if kt == 4 and pending_tail is not None:
                                # previous block's reciprocal+muls emitted
                                # here so the DVE queue isn't head-of-line
                                # blocked on the Pool all_reduce latency
                                pending_tail()
                                pending_tail = None
                        for k in (KT - 2, KT - 1):
                            for i in range(2):
                                nc.tensor.matmul(
                                    opst[:, i, :],
                                    lhsT=Vs[:, k, :],
                                    rhs=pt[:, k, i, :],
                                    start=False,
                                    stop=(k == KT - 1),
                                )
                        acc_p(KT - 1)
                        # acct on DVE (no cross-engine wait after add(15));
                        # only the partition_all_reduce (which broadcasts the
                        # key-sum to all partitions) runs on Pool
                        acct = sm.tile([128, 2, CHS], F32, tag="acct")
                        nc.vector.tensor_add(acct, accd, accp)
                        ar = sm.tile([128, 2, CHS], F32, tag="ar")
                        nc.gpsimd.partition_all_reduce(
                            ar, acct, 128, bass_isa.ReduceOp.add
                        )

                        def mk_tail(opst=opst, ar=ar, jp=jp, h=h):
                            def tail():
                                rec = sm.tile([128, 2, CHS], F32, tag="rec")
                                # ~18 correct bits — plenty for a softmax
                                # denominator
                                nc.vector.reciprocal_approx_fast(rec, ar)
                                for i in range(2):
                                    j = 2 * jp + i
                                    jsl = slice(j * CHS, (j + 1) * CHS)
                                    nc.vector.tensor_mul(
                                        OT[h][:, jsl], opst[:, i, :], rec[:, i, :]
                                    )
                            return tail

                        pending_tail = mk_tail()
                if pending_tail is not None:
                    pending_tail()

                if phases == 2:
                    for h in range(qh):
                        nc.sync.dma_start(
                            out=y[h * 128:(h + 1) * 128, 0:seq], in_=OT[h]
                        )
                    return

                # ---------------- Phase C: output projection ----------------
                # PSUM comes from the still-open sps/aps pools as bank-PAIRS
                # (same tile shape/tag as st/opst) so the only dependencies
                # are tile-level WARs, not pool-release barriers.
                QS = seq // 128
                ECH = dim // 512
                PAIR_TAG = {0: sps, 1: sps, 2: aps, 3: aps}
                pairs = {}      # (qs, pi) -> [128, 2, 512] PSUM pair tile
                ystages = {}    # qs -> staging tile

                def yb(qs, e):
                    pi = e // 2
                    if (qs, pi) not in pairs:
                        pool = PAIR_TAG[pi]
                        tag = "st" if pool is sps else "opst"
                        pairs[(qs, pi)] = pool.tile(
                            [128, 2, CHS], F32, name=f"yb{qs}_{pi}", tag=tag
                        )
                    return pairs[(qs, pi)][:, e % 2, :]

                def half_round(qs, es, drain_eng=None, dma_after=None):
                    """One full h accumulation round over PSUM banks `es`.

                    drain_eng: "act" = ACT-only drains (used while the DVE
                    queue is still busy with phase B's last softmax tail).
                    """
                    if qs not in ystages:
                        ystages[qs] = ysb.tile(
                            [128, dim], BF16, name=f"yt{qs}", tag="yt"
                        )
                    ystage = ystages[qs]
                    outs = {e: yb(qs, e) for e in es}
                    for h in range(qh):
                        for e in es:
                            nc.tensor.matmul(
                                outs[e],
                                lhsT=OT[h][:, qs * 128:(qs + 1) * 128],
                                rhs=wo_r[:, h, e * 512:(e + 1) * 512],
                                start=(h == 0),
                                stop=(h == qh - 1),
                            )
                            if h == qh - 1:
                                esl = slice(e * 512, (e + 1) * 512)
                                act = (
                                    drain_eng == "act"
                                    if drain_eng
                                    else e % 4 != 0
                                )
                                if act:
                                    nc.scalar.activation(
                                        out=ystage[:, esl], in_=outs[e],
                                        func=COPY,
                                    )
                                else:
                                    nc.vector.tensor_copy(
                                        out=ystage[:, esl], in_=outs[e]
                                    )
                                if dma_after is not None and e == dma_after:
                                    lo = es[0] * 512
                                    hi = (e + 1) * 512
                                    nc.sync.dma_start(
                                        out=y[qs * 128:(qs + 1) * 128, lo:hi],
                                        in_=ystage[:, lo:hi],
                                    )

                LO, HI = tuple(range(4)), tuple(range(4, ECH))
                if phases == 4:   # debug: plain v3-style C from shared pools
                    for qs in range(QS):
                        half_round(qs, tuple(range(ECH)))
                        nc.sync.dma_start(out=y[qs * 128:(qs + 1) * 128, :],
                                          in_=ystages[qs])
                    return
                # staged opening: three blocks on the st banks first
                # (ACT-only drains) — phase B's last softmax tail is still
                # draining the last opst banks and the DVE queue
                for qs in range(3):
                    half_round(qs, LO, drain_eng="act")
                for qs in range(3):
                    half_round(qs, HI)
                    nc.sync.dma_start(out=y[qs * 128:(qs + 1) * 128, :],
                                      in_=ystages[qs])
                for qs in range(3, QS):
                    last = qs == QS - 1
                    if last:
                        # half-split so the first half's drains+store overlap
                        # the second half's matmuls; the second half runs
                        # e-outer so each bank's drain+store starts as early
                        # as possible, shortening the end drain
                        half_round(qs, LO, dma_after=3)
                        ystage = ystages[qs]
                        for e in HI:
                            out = yb(qs, e)
                            for h in range(qh):
                                nc.tensor.matmul(
                                    out,
                                    lhsT=OT[h][:, qs * 128:(qs + 1) * 128],
                                    rhs=wo_r[:, h, e * 512:(e + 1) * 512],
                                    start=(h == 0),
                                    stop=(h == qh - 1),
                                )
                            esl = slice(e * 512, (e + 1) * 512)
                            if e % 2 == 0:
                                nc.vector.tensor_copy(out=ystage[:, esl], in_=out)
                            else:
                                nc.scalar.activation(
                                    out=ystage[:, esl], in_=out, func=COPY,
                                )
                            nc.sync.dma_start(
                                out=y[qs * 128:(qs + 1) * 128, esl],
                                in_=ystage[:, esl],
                            )
                    else:
                        # single h round over all 8 banks: 4 stationary
                        # loads per block (each reused 8x)
                        half_round(qs, tuple(range(ECH)))
                        nc.sync.dma_start(out=y[qs * 128:(qs + 1) * 128, :],
                                          in_=ystages[qs])


def build_nc(dim=DIM, seq=SEQ, qh=QH, repeat=1, phases=3):
    ct = dim // 128
    nc = bacc.Bacc("TRN2", target_bir_lowering=False, debug=False)
    xT = nc.dram_tensor("xT", [dim, seq], BF16, kind="ExternalInput").ap()
    wqkv = nc.dram_tensor(
        "wqkv", [ct, 128, (qh + 2) * HEAD_DIM], BF16, kind="ExternalInput"
    ).ap()
    woT = nc.dram_tensor("woT", [qh, HEAD_DIM, dim], BF16, kind="ExternalInput").ap()
    csn = nc.dram_tensor("csn", [2, 128, seq], BF16, kind="ExternalInput").ap()
    y = nc.dram_tensor("y", [seq, dim], BF16, kind="ExternalOutput").ap()
    with tile.TileContext(nc) as tc:
        for _ in range(repeat):
            _body(tc, xT, wqkv, woT, csn, y, dim, seq, qh, phases=phases)
    nc.compile()
    return nc


def make_in_maps(x, freqs, wq, wk, wv, wo, cores=CORES):
    """Host-side sharding: returns list of per-core input dicts."""
    import ml_dtypes

    bf = ml_dtypes.bfloat16
    dim = x.shape[1]
    seq = x.shape[0]
    hd = HEAD_DIM
    n_heads = wq.shape[0] // hd
    n_kv = wk.shape[0] // hd
    qh = n_heads // cores
    ct = dim // 128

    perm = np.concatenate([np.arange(0, hd, 2), np.arange(1, hd, 2)])
    cos = np.cos(freqs).T.astype(np.float32)  # [64, S]
    sin = np.sin(freqs).T.astype(np.float32)
    csn = np.stack(
        [
            np.concatenate([cos, cos], axis=0),
            np.concatenate([-sin, sin], axis=0),
        ]
    ).astype(bf)  # [2, 128, S] bf16

    xT = np.ascontiguousarray(x.T.astype(bf))  # [dim, seq] bf16

    wq_r = wq.reshape(n_heads, hd, dim)
    wk_r = wk.reshape(n_kv, hd, dim)
    wv_r = wv.reshape(n_kv, hd, dim)

    in_maps = []
    for g in range(cores):
        wq_g = wq_r[g * qh:(g + 1) * qh][:, perm, :]  # [qh, 128, dim]
        wk_g = wk_r[g][perm, :]                       # [128, dim]
        wv_g = wv_r[g]                                # [128, dim]
        wq_t = (
            wq_g.reshape(qh, hd, ct, 128).transpose(2, 3, 0, 1).reshape(ct, 128, qh * hd)
        )
        wk_t = wk_g.reshape(hd, ct, 128).transpose(1, 2, 0)  # [ct, 128, 128]
        wv_t = wv_g.reshape(hd, ct, 128).transpose(1, 2, 0)
        wqkv_g = np.ascontiguousarray(
            np.concatenate([wq_t, wk_t, wv_t], axis=2), dtype=bf
        )
        wo_g = wo[:, g * qh * hd:(g + 1) * qh * hd]   # [dim, qh*128]
        woT_g = np.ascontiguousarray(wo_g.T.reshape(qh, hd, dim), dtype=bf)
        in_maps.append({"xT": xT, "wqkv": wqkv_g, "woT": woT_g, "csn": csn})
    return in_maps


_NC_CACHE = {}


def kernel(x, freqs, wq, wk, wv, wo):
    x = np.asarray(x, dtype=np.float32)
    freqs = np.asarray(freqs, dtype=np.float32)
    wq = np.asarray(wq, dtype=np.float32)
    wk = np.asarray(wk, dtype=np.float32)
    wv = np.asarray(wv, dtype=np.float32)
    wo = np.asarray(wo, dtype=np.float32)

    key = (DIM, SEQ, QH)
    if key not in _NC_CACHE:
        _NC_CACHE[key] = build_nc(DIM, SEQ, QH)
    nc = _NC_CACHE[key]

    in_maps = make_in_maps(x, freqs, wq, wk, wv, wo, CORES)
    res = run_bass_kernel_spmd(nc, in_maps, list(range(CORES)))
    parts = [np.asarray(res.results[g]["y"], dtype=np.float32) for g in range(CORES)]
    return np.sum(np.stack(parts), axis=0, dtype=np.float32)


if __name__ == "__main__":
    import reference

    inputs = reference.setup_inputs()
    out = kernel(**{k: np.asarray(v) for k, v in inputs.items()})
    print("kernel out", out.shape, out.dtype)


# revision 48
# speedup vs baseline: 163.5823x; 163.5823x over previous
"""GQA attention (RoPE, full softmax) on 8 TRN2 NeuronCores — v5.

Tensor-parallel over heads: core g owns KV head g and Q heads 4g..4g+3.
Each core computes y_g = concat_h(softmax(Q_h K^T) V_h) @ wo_h^T; the host
sums the 8 bf16 partials in fp32.

Design (TimelineSim-driven; sim 441us vs 534us for the ones-matmul
baseline):
  - softmax row sums are OFF the PE: P tiles accumulate via tensor_add
    (DVE, with kt 3/6/9/12 on the otherwise-idle GpSimd/Pool engine),
    then one Pool partition_all_reduce broadcasts the key-sum to all
    partitions, reciprocal_approx_fast, two muls into OT.  This deletes
    the 256 row-sum ones-matmuls (~57us of PE) and two PSUM banks.
  - each block's reciprocal+muls are software-pipelined into the NEXT
    block (emitted at kt==4) so the DVE queue never head-of-line blocks
    on the Pool all_reduce latency.
  - PSUM: S double-buffered (2x2 banks) + PV accumulator double-buffered
    (2x2 banks); PV lags S by two kt to hide the exp semaphore latency.
    Phase B is jointly PE/ACT-saturated (exp [128,1024] ~1.04us/kt ==
    2 matmuls + ldweights).  A matmul may not cross a PSUM bank boundary.
  - phase A: pass 0 = K/V (their weight columns only — pass 0 is
    DMA-bandwidth-bound), passes 1-2 = Q heads; Q weights and bf16
    cos/sin stream during pass 1.  PSUM drains are emitted inline in the
    last c-iteration (pass-2 drains mostly on DVE so phase B's first exp
    isn't queued behind them); rope runs all-bf16 on the DVE fast path.
    The cos/sin DMAs MUST be emitted before post_pass(0) reads them —
    consumers emitted before producers get NO dependency (HW race).
  - pools: pool-release boundaries depend on every pool user, so the
    phase-B pools stay open through phase C, whose PSUM banks are
    allocated from the same st/opst tags (tile-level WAR only).  pp/sm
    open before wo/OT/ysb so they land on SBUF freed by matmul-only
    readers, not the rope-dependent stage regions.
  - phase C opens with three qs blocks on the st banks (ACT-only drains)
    while the last softmax tail drains; the last block runs its second
    half e-outer with per-bank stores to shorten the end drain.

fp32 PSUM accumulation everywhere; bf16 weights/activations/rope tables
keep rel err ~8e-3, under the 2e-2 gate with 2.4x margin.  exp() without
max-subtraction is safe: scores ~ N(0,1), |s|max ~ 8.
"""

import numpy as np

import concourse.bass as bass
import concourse.bass_isa as bass_isa
import concourse.mybir as mybir
import concourse.tile as tile
from concourse import bacc
from concourse.bass_utils import run_bass_kernel_spmd
from concourse.masks import make_identity

F32 = mybir.dt.float32
BF16 = mybir.dt.bfloat16
EXP = mybir.ActivationFunctionType.Exp
COPY = mybir.ActivationFunctionType.Copy

DIM, N_HEADS, N_KV_HEADS, HEAD_DIM, SEQ = 4096, 32, 8, 128, 2048
CORES = 8
QH = N_HEADS // CORES  # q heads per core
CHS = 512              # q-chunk size (1 PSUM bank of fp32)
RES = 17               # x c-tiles kept SBUF-resident across phase-A passes


def _body(tc, xT, wqkv, woT, csn, y, dim, seq, qh, phases=3):
    nc = tc.nc
    CT = dim // 128   # contraction tiles over model dim
    KT = seq // 128   # key tiles
    CH = seq // CHS   # q chunks
    HD = HEAD_DIM
    scale = HD ** -0.5

    with tc.tile_pool(name="persist", bufs=1) as persist:
        QT = [persist.tile([128, seq], BF16, name=f"qt{h}", tag=f"qt{h}") for h in range(qh)]
        KTs = persist.tile([128, seq], BF16, tag="kts")
        Vs = persist.tile([128, KT, HD], BF16, tag="vs")
        ident = persist.tile([128, 128], F32, tag="ident")
        make_identity(nc, ident)

        # ------------- Phase A: QKV projections + RoPE + V transpose -------------
        # passes: 0 -> {k,v}, 1 -> {q0,q1}, 2 -> {q2,q3}
        with (
            tc.tile_pool(name="xres", bufs=1) as xres,
            tc.tile_pool(name="wqa", bufs=1) as wqa,
            tc.tile_pool(name="csnp", bufs=1) as csnp,
            tc.tile_pool(name="xs", bufs=4) as xs,
            tc.tile_pool(name="stage", bufs=1) as stg,
            tc.tile_pool(name="rope", bufs=2) as rp,
            tc.tile_pool(name="pps", bufs=8, space="PSUM") as pps,
        ):
            cs_t = csnp.tile([128, seq], BF16, tag="cs")
            sn_t = csnp.tile([128, seq], BF16, tag="sn")
            xresid = xres.tile([128, RES, seq], BF16, tag="xres")
            wqall = wqa.tile([128, CT, (qh + 2) * HD], BF16, tag="wqall")
            KVC = slice(qh * HD, (qh + 2) * HD)   # K,V weight columns

            def w_dma(csl, nsl):
                nc.sync.dma_start(
                    out=wqall[:, csl, nsl],
                    in_=wqkv[csl, :, nsl].rearrange("c p n -> p c n"),
                )

            # startup ramp: pass 0 only needs the K/V weight columns (2.1MB
            # instead of 6.3MB) — pass 0 is DMA-bandwidth-bound (16MB of x),
            # so Q weights + cos/sin stream during pass 1 instead.
            # first x tile arrives in j-sized chunks: matmul (c=0, j=0) only
            # needs columns 0:512, so it starts ~2us earlier
            nc.sync.dma_start(out=xresid[:, 0, 0:CHS], in_=xT[0:128, 0:CHS])
            w_dma(slice(0, 1), KVC)
            nc.sync.dma_start(out=xresid[:, 0, CHS:], in_=xT[0:128, CHS:])
            nc.sync.dma_start(out=xresid[:, 1, :], in_=xT[128:256, :])
            w_dma(slice(1, 4), KVC)
            nc.sync.dma_start(out=xresid[:, 2, :], in_=xT[256:384, :])
            w_dma(slice(4, 8), KVC)

            def drain(p, t, j, ps, tsbs, vts):
                """PSUM->SBUF drain for out-tile (t, j), emitted inline right
                after its final (stop=True) matmul so banks free during the
                last c-iteration instead of after the pass."""
                if p == 0 and t == 1:
                    nc.scalar.activation(out=vts[j], in_=ps[1][j], func=COPY)
                elif p == 1 and t == 1:
                    nc.scalar.activation(out=tsbs[j][:, 1, :], in_=ps[1][j], func=COPY)
                elif p == 2 and t == 0 and j < 2:
                    # pass 2: only the two earliest-stopping tiles on ACT
                    # (they finish before phase B's first exp is reached);
                    # the rest on DVE.  8 all-DVE drains spill ~2us past the
                    # pass; 6 fit inside the last c-iteration's matmul window
                    nc.scalar.activation(out=tsbs[j][:, 0, :], in_=ps[0][j], func=COPY)
                else:
                    nc.vector.tensor_copy(out=tsbs[j][:, t, :], in_=ps[t][j])

            def post_pass(p, ps, tsbs, vts):
                """Swaps, V transposes, then the rope mul chains (the PSUM
                drains already ran inline in the last c-iteration)."""
                sws = []
                m = 1 if p == 0 else 2
                for j in range(CH):
                    sw = stg.tile([128, 2, CHS], BF16, name=f"sw{j}", tag=f"sw{j}")
                    sws.append(sw)
                    nc.sync.dma_start(out=sw[0:64, 0:m], in_=tsbs[j][64:128, 0:m])
                    nc.sync.dma_start(out=sw[64:128, 0:m], in_=tsbs[j][0:64, 0:m])
                if p == 0:
                    # V transpose: [d, keys] -> [keys, d] per 128-key block
                    for j in range(CH):
                        for b in range(CHS // HD):
                            trp = pps.tile([128, HD], F32, name="trp", tag="mm")
                            nc.tensor.transpose(
                                trp, vts[j][:, b * HD:(b + 1) * HD], ident
                            )
                            nc.vector.tensor_copy(
                                out=Vs[:, j * (CHS // HD) + b, :], in_=trp
                            )
                for j in range(CH):
                    jsl = slice(j * CHS, (j + 1) * CHS)
                    outs = (
                        [KTs[:, jsl]] if p == 0
                        else [QT[2 * (p - 1)][:, jsl], QT[2 * p - 1][:, jsl]]
                    )
                    for i, out in enumerate(outs):
                        # all-bf16 operands keep these on the DVE 2x/4x path
                        t1 = rp.tile([128, CHS], BF16, tag="t1")
                        t2 = rp.tile([128, CHS], BF16, tag="t2")
                        nc.vector.tensor_mul(t1, tsbs[j][:, i, :], cs_t[:, jsl])
                        nc.vector.tensor_mul(t2, sws[j][:, i, :], sn_t[:, jsl])
                        nc.vector.tensor_add(out, t1, t2)

            for p in range(3):
                ps = [
                    [
                        pps.tile([128, CHS], F32, name=f"mm{t}{j}", tag="mm")
                        for j in range(CH)
                    ]
                    for t in range(2)
                ]
                for c in range(CT):
                    if p == 0:
                        # remaining K/V weight c-tiles mid-pass; the first Q
                        # c-tiles late in the pass (pass 1 needs them at its
                        # c=0); everything else during pass 1, whose DMA
                        # queue is nearly idle until c reaches RES
                        if c == 6:
                            w_dma(slice(8, 20), KVC)
                        elif c == 14:
                            w_dma(slice(20, CT), KVC)
                        elif c >= 28:
                            w_dma(slice(c - 28, c - 27), slice(0, 2 * HD))
                    if p == 1:
                        if c == 0:
                            w_dma(slice(4, 8), slice(0, 2 * HD))
                        elif c == 2:
                            w_dma(slice(8, 16), slice(0, 2 * HD))
                        elif c == 4:
                            w_dma(slice(16, 24), slice(0, 2 * HD))
                        elif c == 6:
                            w_dma(slice(24, CT), slice(0, 2 * HD))
                        elif c == 8:
                            w_dma(slice(0, 16), slice(2 * HD, 4 * HD))
                        elif c == 16:
                            w_dma(slice(16, CT), slice(2 * HD, 4 * HD))
                    if c < RES:
                        xt = xresid[:, c, :]
                        if p == 0 and c >= 3:
                            nc.sync.dma_start(out=xt, in_=xT[c * 128:(c + 1) * 128, :])
                    else:
                        xt = xs.tile([128, seq], BF16, name="xt", tag="xs")
                        nc.sync.dma_start(out=xt, in_=xT[c * 128:(c + 1) * 128, :])
                    if c == CT - 1:
                        tsbs = [
                            stg.tile([128, 2, CHS], BF16, name=f"tsb{j}", tag=f"tsb{j}")
                            for j in range(CH)
                        ]
                        vts = (
                            [
                                stg.tile([128, CHS], F32, name=f"vt{j}", tag=f"vt{j}")
                                for j in range(CH)
                            ]
                            if p == 0
                            else None
                        )
                    for t in range(2):
                        tile_idx = (4 + t) if p == 0 else (2 * (p - 1) + t)
                        w_sl = slice(tile_idx * HD, (tile_idx + 1) * HD)
                        for j in range(CH):
                            nc.tensor.matmul(
                                ps[t][j],
                                lhsT=wqall[:, c, w_sl],
                                rhs=xt[:, j * CHS:(j + 1) * CHS],
                                start=(c == 0),
                                stop=(c == CT - 1),
                            )
                            if c == CT - 1:
                                drain(p, t, j, ps, tsbs, vts)
                if p == 0:
                    # MUST be emitted before post_pass(0)'s rope muls read
                    # them (consumers emitted before producers get no
                    # dependency); queued after pass 0's x streams so the
                    # pass-0 DMA budget is unaffected
                    nc.sync.dma_start(out=cs_t, in_=csn[0])
                    nc.sync.dma_start(out=sn_t, in_=csn[1])
                post_pass(p, ps, tsbs, vts)

        if phases == 1:
            nc.sync.dma_start(out=y[0:128, 0:seq], in_=KTs)
            return

        # ---------------- Phase B: attention per (chunk-pair, head) ----------------
        # NOTE: the B pools stay open through phase C — a pool release
        # boundary depends on EVERY user of the pool, so closing them would
        # serialize C's PSUM allocation behind B's whole softmax tail.
        # Phase C instead allocates its PSUM banks from the same st/opst
        # tags, whose WAR dependencies resolve tile-by-tile.
        # pool order matters: pp/sm open FIRST so they land on the SBUF
        # regions freed by xres/wqa (whose last users are plain matmuls,
        # done at phase-A end) — NOT on the stage/rope regions whose
        # release waits for the whole pass-2 rope chain on DVE.  wo/OT/ysb
        # land on the rope-dependent regions; their first uses are late.
        with (
            tc.tile_pool(name="pp", bufs=2) as pp,
            tc.tile_pool(name="sm", bufs=2) as sm,
            tc.tile_pool(name="sps", bufs=2, space="PSUM") as sps,
            tc.tile_pool(name="aps", bufs=2, space="PSUM") as aps,
            tc.tile_pool(name="wo", bufs=1) as wop,
            tc.tile_pool(name="otp", bufs=1) as otp,
            tc.tile_pool(name="ysb", bufs=3) as ysb,
        ):
            # preload wo during phase B (used in phase C)
            wo_r = wop.tile([128, qh, dim], BF16)
            for h in range(qh):
                nc.sync.dma_start(out=wo_r[:, h, :], in_=woT[h])
            OT = [otp.tile([128, seq], BF16, name=f"ot{h}", tag=f"ot{h}") for h in range(qh)]
            if True:
                pending_tail = None
                for jp in range(CH // 2):
                    for h in range(qh):
                        pt = pp.tile([128, KT, 2, CHS], BF16, name="pt", tag="pt")
                        opst = aps.tile([128, 2, CHS], F32, name="opst", tag="opst")
                        accd = sm.tile([128, 2, CHS], F32, tag="accd")
                        accp = sm.tile([128, 2, CHS], F32, tag="accp")

                        def acc_p(k, pt=pt, accd=accd, accp=accp):
                            # row-sum accumulation off the PE.  Pool adds are
                            # ~2.1us/[128,1024] (software engine) vs DVE's
                            # ~0.85us, so Pool only takes every 4th kt —
                            # leaving Pool idle at block end so the
                            # all_reduce starts immediately.
                            pool = k in (3, 6, 9, 12)
                            eng = nc.gpsimd if pool else nc.vector
                            acc = accp if pool else accd
                            first = k == 3 if pool else k == 0
                            if first:
                                eng.tensor_copy(out=acc, in_=pt[:, k])
                            else:
                                eng.tensor_add(acc, acc, pt[:, k])

                        for kt in range(KT):
                            st = sps.tile([128, 2, CHS], F32, name="st", tag="st")
                            # a matmul may not cross a PSUM bank boundary
                            # (CoreSim-verified), so S stays two FD=512 ops
                            for i in range(2):
                                j = 2 * jp + i
                                nc.tensor.matmul(
                                    st[:, i, :],
                                    lhsT=KTs[:, kt * 128:(kt + 1) * 128],
                                    rhs=QT[h][:, j * CHS:(j + 1) * CHS],
                                    start=True,
                                    stop=True,
                                )
                            nc.scalar.activation(
                                out=pt[:, kt], in_=st, func=EXP, scale=scale
                            )
                            # PV lags TWO kt so the in-order PE queue never
                            # sees the exp semaphore-settle latency
                            if kt > 1:
                                for i in range(2):
                                    nc.tensor.matmul(
                                        opst[:, i, :],
                                        lhsT=Vs[:, kt - 2, :],
                                        rhs=pt[:, kt - 2, i, :],
                                        start=(kt - 2 == 0),
                                        stop=False,
                                    )
                            if kt > 0:
                                acc_p(kt - 1)
                            if kt == 4 and pending_tail is not None:
                                # previous block's reciprocal+muls emitted
                                # here so the DVE queue isn't head-of-line
                                # blocked on the Pool all_reduce latency
                                pending_tail()
                                pending_tail = None
                        for k in (KT - 2, KT - 1):
                            for i in range(2):
                                nc.tensor.matmul(
                                    opst[:, i, :],
                                    lhsT=Vs[:, k, :],
                                    rhs=pt[:, k, i, :],
                                    start=False,
                                    stop=(k == KT - 1),
                                )
                        acc_p(KT - 1)
                        # acct on DVE (no cross-engine wait after add(15));
                        # only the partition_all_reduce (which broadcasts the
                        # key-sum to all partitions) runs on Pool
                        acct = sm.tile([128, 2, CHS], F32, tag="acct")
                        nc.vector.tensor_add(acct, accd, accp)
                        ar = sm.tile([128, 2, CHS], F32, tag="ar")
                        nc.gpsimd.partition_all_reduce(
                            ar, acct, 128, bass_isa.ReduceOp.add
                        )

                        def mk_tail(opst=opst, ar=ar, jp=jp, h=h):
                            def tail():
                                rec = sm.tile([128, 2, CHS], F32, tag="rec")
                                # ~18 correct bits — plenty for a softmax
                                # denominator
                                nc.vector.reciprocal_approx_fast(rec, ar)
                                for i in range(2):
                                    j = 2 * jp + i
                                    jsl = slice(j * CHS, (j + 1) * CHS)
                                    nc.vector.tensor_mul(
                                        OT[h][:, jsl], opst[:, i, :], rec[:, i, :]
                                    )
                            return tail

                        pending_tail = mk_tail()
                if pending_tail is not None:
                    pending_tail()

                if phases == 2:
                    for h in range(qh):
                        nc.sync.dma_start(
                            out=y[h * 128:(h + 1) * 128, 0:seq], in_=OT[h]
                        )
                    return

                # ---------------- Phase C: output projection ----------------
                # PSUM comes from the still-open sps/aps pools as bank-PAIRS
                # (same tile shape/tag as st/opst) so the only dependencies
                # are tile-level WARs, not pool-release barriers.
                QS = seq // 128
                ECH = dim // 512
                PAIR_TAG = {0: sps, 1: sps, 2: aps, 3: aps}
                pairs = {}      # (qs, pi) -> [128, 2, 512] PSUM pair tile
                ystages = {}    # qs -> staging tile

                def yb(qs, e):
                    pi = e // 2
                    if (qs, pi) not in pairs:
                        pool = PAIR_TAG[pi]
                        tag = "st" if pool is sps else "opst"
                        pairs[(qs, pi)] = pool.tile(
                            [128, 2, CHS], F32, name=f"yb{qs}_{pi}", tag=tag
                        )
                    return pairs[(qs, pi)][:, e % 2, :]

                def half_round(qs, es, drain_eng=None, dma_after=None):
                    """One full h accumulation round over PSUM banks `es`.

                    drain_eng: "act" = ACT-only drains (used while the DVE
                    queue is still busy with phase B's last softmax tail).
                    """
                    if qs not in ystages:
                        ystages[qs] = ysb.tile(
                            [128, dim], BF16, name=f"yt{qs}", tag="yt"
                        )
                    ystage = ystages[qs]
                    outs = {e: yb(qs, e) for e in es}
                    for h in range(qh):
                        for e in es:
                            nc.tensor.matmul(
                                outs[e],
                                lhsT=OT[h][:, qs * 128:(qs + 1) * 128],
                                rhs=wo_r[:, h, e * 512:(e + 1) * 512],
                                start=(h == 0),
                                stop=(h == qh - 1),
                            )
                            if h == qh - 1:
                                esl = slice(e * 512, (e + 1) * 512)
                                act = (
                                    drain_eng == "act"
                                    if drain_eng
                                    else e % 4 != 0
                                )
                                if act:
                                    nc.scalar.activation(
                                        out=ystage[:, esl], in_=outs[e],
                                        func=COPY,
                                    )
                                else:
                                    nc.vector.tensor_copy(
                                        out=ystage[:, esl], in_=outs[e]
                                    )
                                if dma_after is not None and e == dma_after:
                                    lo = es[0] * 512
                                    hi = (e + 1) * 512
                                    nc.sync.dma_start(
                                        out=y[qs * 128:(qs + 1) * 128, lo:hi],
                                        in_=ystage[:, lo:hi],
                                    )

                LO, HI = tuple(range(4)), tuple(range(4, ECH))
                if phases == 4:   # debug: plain v3-style C from shared pools
                    for qs in range(QS):
                        half_round(qs, tuple(range(ECH)))
                        nc.sync.dma_start(out=y[qs * 128:(qs + 1) * 128, :],
                                          in_=ystages[qs])
                    return
                # staged opening: three blocks on the st banks first
                # (ACT-only drains) — phase B's last softmax tail is still
                # draining the last opst banks and the DVE queue
                for qs in range(3):
                    half_round(qs, LO, drain_eng="act")
                for qs in range(3):
                    half_round(qs, HI)
                    nc.sync.dma_start(out=y[qs * 128:(qs + 1) * 128, :],
                                      in_=ystages[qs])
                for qs in range(3, QS):
                    last = qs == QS - 1
                    if last:
                        # half-split so the first half's drains+store overlap
                        # the second half's matmuls; the second half runs
                        # e-outer so each bank's drain+store starts as early
                        # as possible, shortening the end drain
                        half_round(qs, LO, dma_after=3)
                        ystage = ystages[qs]
                        for e in reversed(HI):
                            out = yb(qs, e)
                            for h in range(qh):
                                nc.tensor.matmul(
                                    out,
                                    lhsT=OT[h][:, qs * 128:(qs + 1) * 128],
                                    rhs=wo_r[:, h, e * 512:(e + 1) * 512],
                                    start=(h == 0),
                                    stop=(h == qh - 1),
                                )
                            esl = slice(e * 512, (e + 1) * 512)
                            if e % 2 == 0:
                                nc.vector.tensor_copy(out=ystage[:, esl], in_=out)
                            else:
                                nc.scalar.activation(
                                    out=ystage[:, esl], in_=out, func=COPY,
                                )
                            nc.sync.dma_start(
                                out=y[qs * 128:(qs + 1) * 128, esl],
                                in_=ystage[:, esl],
                            )
                    else:
                        # single h round over all 8 banks: 4 stationary
                        # loads per block (each reused 8x)
                        half_round(qs, tuple(range(ECH)))
                        nc.sync.dma_start(out=y[qs * 128:(qs + 1) * 128, :],
                                          in_=ystages[qs])


def build_nc(dim=DIM, seq=SEQ, qh=QH, repeat=1, phases=3):
    ct = dim // 128
    nc = bacc.Bacc("TRN2", target_bir_lowering=False, debug=False)
    xT = nc.dram_tensor("xT", [dim, seq], BF16, kind="ExternalInput").ap()
    wqkv = nc.dram_tensor(
        "wqkv", [ct, 128, (qh + 2) * HEAD_DIM], BF16, kind="ExternalInput"
    ).ap()
    woT = nc.dram_tensor("woT", [qh, HEAD_DIM, dim], BF16, kind="ExternalInput").ap()
    csn = nc.dram_tensor("csn", [2, 128, seq], BF16, kind="ExternalInput").ap()
    y = nc.dram_tensor("y", [seq, dim], BF16, kind="ExternalOutput").ap()
    with tile.TileContext(nc) as tc:
        for _ in range(repeat):
            _body(tc, xT, wqkv, woT, csn, y, dim, seq, qh, phases=phases)
    nc.compile()
    return nc


def make_in_maps(x, freqs, wq, wk, wv, wo, cores=CORES):
    """Host-side sharding: returns list of per-core input dicts."""
    import ml_dtypes

    bf = ml_dtypes.bfloat16
    dim = x.shape[1]
    seq = x.shape[0]
    hd = HEAD_DIM
    n_heads = wq.shape[0] // hd
    n_kv = wk.shape[0] // hd
    qh = n_heads // cores
    ct = dim // 128

    perm = np.concatenate([np.arange(0, hd, 2), np.arange(1, hd, 2)])
    cos = np.cos(freqs).T.astype(np.float32)  # [64, S]
    sin = np.sin(freqs).T.astype(np.float32)
    csn = np.stack(
        [
            np.concatenate([cos, cos], axis=0),
            np.concatenate([-sin, sin], axis=0),
        ]
    ).astype(bf)  # [2, 128, S] bf16

    xT = np.ascontiguousarray(x.T.astype(bf))  # [dim, seq] bf16

    wq_r = wq.reshape(n_heads, hd, dim)
    wk_r = wk.reshape(n_kv, hd, dim)
    wv_r = wv.reshape(n_kv, hd, dim)

    in_maps = []
    for g in range(cores):
        wq_g = wq_r[g * qh:(g + 1) * qh][:, perm, :]  # [qh, 128, dim]
        wk_g = wk_r[g][perm, :]                       # [128, dim]
        wv_g = wv_r[g]                                # [128, dim]
        wq_t = (
            wq_g.reshape(qh, hd, ct, 128).transpose(2, 3, 0, 1).reshape(ct, 128, qh * hd)
        )
        wk_t = wk_g.reshape(hd, ct, 128).transpose(1, 2, 0)  # [ct, 128, 128]
        wv_t = wv_g.reshape(hd, ct, 128).transpose(1, 2, 0)
        wqkv_g = np.ascontiguousarray(
            np.concatenate([wq_t, wk_t, wv_t], axis=2), dtype=bf
        )
        wo_g = wo[:, g * qh * hd:(g + 1) * qh * hd]   # [dim, qh*128]
        woT_g = np.ascontiguousarray(wo_g.T.reshape(qh, hd, dim), dtype=bf)
        in_maps.append({"xT": xT, "wqkv": wqkv_g, "woT": woT_g, "csn": csn})
    return in_maps


_NC_CACHE = {}


def kernel(x, freqs, wq, wk, wv, wo):
    x = np.asarray(x, dtype=np.float32)
    freqs = np.asarray(freqs, dtype=np.float32)
    wq = np.asarray(wq, dtype=np.float32)
    wk = np.asarray(wk, dtype=np.float32)
    wv = np.asarray(wv, dtype=np.float32)
    wo = np.asarray(wo, dtype=np.float32)

    key = (DIM, SEQ, QH)
    if key not in _NC_CACHE:
        _NC_CACHE[key] = build_nc(DIM, SEQ, QH)
    nc = _NC_CACHE[key]

    in_maps = make_in_maps(x, freqs, wq, wk, wv, wo, CORES)
    res = run_bass_kernel_spmd(nc, in_maps, list(range(CORES)))
    parts = [np.asarray(res.results[g]["y"], dtype=np.float32) for g in range(CORES)]
    return np.sum(np.stack(parts), axis=0, dtype=np.float32)


if __name__ == "__main__":
    import reference

    inputs = reference.setup_inputs()
    out = kernel(**{k: np.asarray(v) for k, v in inputs.items()})
    print("kernel out", out.shape, out.dtype)
